# revision 3
# baseline (speedup 1.0000x reference)
"""Trainium2 Bass kernel for the LSTMGenerator-Diffusion problem.

Math (reference collapses for L=2, T=128):
  h0 = tanh(resnet_h(0.3*noise_init)); c0 = tanh(resnet_c(0.3*noise_init))
  x  = z.reshape(B,2) where z = noise_start_seq_z[:,:,0]
  (h1,c1) = lstm0(x, h0[:, :1024], c0[:, :1024])
  (h2,c2) = lstm1(h1, h0[:, 1024:], c0[:, 1024:])
  dec = tanh(h2) @ dec_w.T                      # (B,1)
  out126 = (z - k1*dec)/sqrt(a) = k2*z - k1*k2*dec
  output (128, B, 2, 1): [127]=z, [126]=out126, rest zeros.

Sharding: pure data-parallel over batch, 8 cores x 4096 rows.
On-device layout: feature-major (features on partitions, batch on the free
dim), 8 batch blocks of 512 columns per core. All matmuls fp32r (full fp32
data, 1 cycle/row at N=512).
"""

import os

os.environ.setdefault("JAX_PLATFORMS", "axon,cpu")

import numpy as np

import concourse.bacc as bacc
import concourse.mybir as mybir
from concourse.tile import TileContext
from concourse.bass_utils import run_bass_kernel_spmd

F32 = mybir.dt.float32
F32R = mybir.dt.float32r
AF = mybir.ActivationFunctionType
OP = mybir.AluOpType

P = 128          # partitions
N = 512          # batch block (free dim)
NBLK = 8         # blocks per core
BC = N * NBLK    # batch per core = 4096
NCORES = 8
B = BC * NCORES  # 32768
H = 1024         # hidden
KH = H // P      # 8 k-chunks of hidden
G = 4 * H        # gate width 4096
T_STEPS = 128

# bias tile columns: [b0h(8) b1h(8) b2h(16) b0c(8) b1c(8) b2c(16) bg0(32) bg1(32) alphas(4) k2(1)]
_C_B0H, _C_B1H, _C_B2H = 0, 8, 16
_C_B0C, _C_B1C, _C_B2C = 32, 40, 48
_C_BG0, _C_BG1 = 64, 96
_C_ALPHA = 128  # a0h, a1h, a0c, a1c
_C_K2 = 132
BIAS_COLS = 133


def _r(ap):
    return ap.bitcast(F32R)


def _build_nc():
    nc = bacc.Bacc(None, target_bir_lowering=False)

    # ---------------- DRAM I/O ----------------
    di = lambda name, shape: nc.dram_tensor(name, shape, F32R, kind="ExternalInput")
    x0_d = di("x0", [NBLK, 2, N])          # z transposed, per block
    nin_d = di("nin", [NBLK, 2, N])        # 0.3*noise_init transposed
    w0h_d = di("w0h", [2, H])
    w0c_d = di("w0c", [2, H])
    w1h_d = di("w1h", [KH, P, H])
    w1c_d = di("w1c", [KH, P, H])
    w2h_d = di("w2h", [KH, P, 2 * H])
    w2c_d = di("w2c", [KH, P, 2 * H])
    wi0_d = di("wi0", [2, G])
    wh0_d = di("wh0", [KH, P, G])
    wi1_d = di("wi1", [KH, P, G])
    wh1_d = di("wh1", [KH, P, G])
    decw_d = di("decw", [KH, P, 2])        # dec weights (pre-scaled), duplicated col
    bias_d = nc.dram_tensor("biases", [P, BIAS_COLS], F32, kind="ExternalInput")

    out_d = nc.dram_tensor("out", [NBLK, 2, N], F32, kind="ExternalOutput")

    # internal DRAM bounce tensors, block-major contiguous (128,512) tiles
    sc = lambda name, dt=F32R: nc.dram_tensor(name, [NBLK, KH, P, N], dt)
    h0l0_d = sc("h0l0")
    h0l1_d = sc("h0l1")
    c0l0_d = sc("c0l0")
    c0l1_d = sc("c0l1")
    x1_d = sc("x1")
    si_d = sc("si", F32)
    pfc_d = sc("pfc", F32)

    with TileContext(nc) as tc:
        with tc.tile_pool(name="bias", bufs=1) as bias_pool:
            bias_sb = bias_pool.tile([P, BIAS_COLS], F32)
            nc.sync.dma_start(out=bias_sb[:], in_=bias_d[:])

            def bcol(c):
                return bias_sb[:, c : c + 1]

            # ---------------- Phase A: resnets (h and c) ----------------
            for net, (w0d, w1d, w2d, cb0, cb1, cb2, ca, ol0, ol1) in enumerate(
                [
                    (w0h_d, w1h_d, w2h_d, _C_B0H, _C_B1H, _C_B2H, _C_ALPHA, h0l0_d, h0l1_d),
                    (w0c_d, w1c_d, w2c_d, _C_B0C, _C_B1C, _C_B2C, _C_ALPHA + 2, c0l0_d, c0l1_d),
                ]
            ):
                with (
                    tc.tile_pool(name=f"wA{net}", bufs=1) as wp,
                    tc.tile_pool(name=f"aA{net}", bufs=2) as apool,
                    tc.tile_pool(name=f"sA{net}", bufs=6) as spool,
                    tc.tile_pool(name=f"pA{net}", bufs=2, space="PSUM") as pp0,
                    tc.tile_pool(name=f"pB{net}", bufs=2, space="PSUM") as pp1,
                    tc.tile_pool(name=f"pC{net}", bufs=4, space="PSUM") as pp2,
                ):
                    w0s = wp.tile([2, H], F32, tag="w0")
                    nc.gpsimd.dma_start(out=_r(w0s[:]), in_=w0d[:])
                    w1s = wp.tile([P, KH * H], F32, tag="w1")
                    for k in range(KH):
                        nc.gpsimd.dma_start(
                            out=_r(w1s[:, k * H : (k + 1) * H]), in_=w1d[k]
                        )
                    w2s = wp.tile([P, KH * 2 * H], F32, tag="w2")
                    for k in range(KH):
                        nc.gpsimd.dma_start(
                            out=_r(w2s[:, k * 2 * H : (k + 1) * 2 * H]), in_=w2d[k]
                        )

                    for i in range(NBLK):
                        nin_t = apool.tile([2, N], F32, tag="nin")
                        nc.sync.dma_start(out=_r(nin_t[:]), in_=nin_d[i])
                        y0s = apool.tile([P, KH * N], F32, tag="y0")
                        y1s = apool.tile([P, KH * N], F32, tag="y1")
                        # L0: y0 = prelu(w0.T @ nin + b0)
                        for m in range(KH):
                            ps = pp0.tile([P, N], F32, tag="ps0")
                            nc.tensor.matmul(
                                ps[:],
                                _r(w0s[:, m * P : (m + 1) * P]),
                                _r(nin_t[:]),
                                start=True,
                                stop=True,
                            )
                            nc.scalar.activation(
                                _r(y0s[:, m * N : (m + 1) * N]),
                                ps[:],
                                AF.Prelu,
                                bias=bcol(cb0 + m),
                                alpha=bcol(ca),
                            )
                        # L1: y1 = y0 + prelu(w1.T @ y0 + b1)
                        for m in range(KH):
                            ps = pp1.tile([P, N], F32, tag="ps1")
                            for k in range(KH):
                                nc.tensor.matmul(
                                    ps[:],
                                    _r(w1s[:, k * H + m * P : k * H + (m + 1) * P]),
                                    _r(y0s[:, k * N : (k + 1) * N]),
                                    start=(k == 0),
                                    stop=(k == KH - 1),
                                )
                            pr = spool.tile([P, N], F32, tag="prelu")
                            nc.scalar.activation(
                                pr[:], ps[:], AF.Prelu, bias=bcol(cb1 + m), alpha=bcol(ca + 1)
                            )
                            nc.vector.tensor_tensor(
                                _r(y1s[:, m * N : (m + 1) * N]),
                                y0s[:, m * N : (m + 1) * N],
                                pr[:],
                                OP.add,
                            )
                        # L2: h0 = tanh(w2.T @ y1 + b2), 16 out chunks
                        for m in range(2 * KH):
                            ps = pp2.tile([P, N], F32, tag="ps2")
                            for k in range(KH):
                                nc.tensor.matmul(
                                    ps[:],
                                    _r(
                                        w2s[
                                            :,
                                            k * 2 * H + m * P : k * 2 * H + (m + 1) * P,
                                        ]
                                    ),
                                    _r(y1s[:, k * N : (k + 1) * N]),
                                    start=(k == 0),
                                    stop=(k == KH - 1),
                                )
                            hs = spool.tile([P, N], F32, tag="hstage")
                            nc.scalar.activation(
                                _r(hs[:]), ps[:], AF.Tanh, bias=bcol(cb2 + m)
                            )
                            dst = ol0 if m < KH else ol1
                            nc.sync.dma_start(out=dst[i, m % KH], in_=_r(hs[:]))

            # ---------------- Phase C: LSTM layer 0 ----------------
            GATE_FUNCS = [AF.Sigmoid, AF.Sigmoid, AF.Tanh, AF.Sigmoid]  # i, f, g, o
            with (
                tc.tile_pool(name="wC", bufs=1) as wp,
                tc.tile_pool(name="aC", bufs=1) as apool,
                tc.tile_pool(name="cC", bufs=2) as cpool,
                tc.tile_pool(name="sC", bufs=2) as spool,
                tc.tile_pool(name="pC", bufs=2, space="PSUM") as pp,
            ):
                wi0s = wp.tile([2, G], F32, tag="wi0")
                nc.gpsimd.dma_start(out=_r(wi0s[:]), in_=wi0_d[:])
                wh0s = wp.tile([P, KH * G], F32, tag="wh0")
                for k in range(KH):
                    nc.gpsimd.dma_start(
                        out=_r(wh0s[:, k * G : (k + 1) * G]), in_=wh0_d[k]
                    )

                for i in range(NBLK):
                    x0_t = apool.tile([2, N], F32, tag="x0")
                    nc.sync.dma_start(out=_r(x0_t[:]), in_=x0_d[i])
                    hp = apool.tile([P, KH * N], F32, tag="hprev")
                    for k in range(KH):
                        nc.sync.dma_start(
                            out=_r(hp[:, k * N : (k + 1) * N]), in_=h0l0_d[i, k]
                        )
                    for j in range(KH):
                        gt = []
                        for gi in range(4):
                            m = gi * KH + j
                            ps = pp.tile([P, N], F32, tag=f"g{gi}")
                            nc.tensor.matmul(
                                ps[:],
                                _r(wi0s[:, m * P : (m + 1) * P]),
                                _r(x0_t[:]),
                                start=True,
                                stop=False,
                            )
                            for k in range(KH):
                                nc.tensor.matmul(
                                    ps[:],
                                    _r(wh0s[:, k * G + m * P : k * G + (m + 1) * P]),
                                    _r(hp[:, k * N : (k + 1) * N]),
                                    start=False,
                                    stop=(k == KH - 1),
                                )
                            g_sb = spool.tile([P, N], F32, tag=f"gs{gi}")
                            nc.scalar.activation(
                                g_sb[:], ps[:], GATE_FUNCS[gi], bias=bcol(_C_BG0 + m)
                            )
                            gt.append(g_sb)
                        # c_new = sf*c + si*tg ; x1 = so*tanh(c_new)
                        ct = cpool.tile([P, N], F32, tag="c0")
                        nc.sync.dma_start(out=_r(ct[:]), in_=c0l0_d[i, j])
                        t1 = spool.tile([P, N], F32, tag="t1")
                        nc.vector.tensor_tensor(t1[:], gt[0][:], gt[2][:], OP.mult)
                        cn = spool.tile([P, N], F32, tag="cn")
                        nc.vector.tensor_tensor(cn[:], gt[1][:], ct[:], OP.mult)
                        nc.vector.tensor_tensor(cn[:], cn[:], t1[:], OP.add)
                        tcn = spool.tile([P, N], F32, tag="tcn")
                        nc.scalar.activation(tcn[:], cn[:], AF.Tanh)
                        xst = cpool.tile([P, N], F32, tag="xst")
                        nc.vector.tensor_tensor(_r(xst[:]), gt[3][:], tcn[:], OP.mult)
                        nc.sync.dma_start(out=x1_d[i, j], in_=_r(xst[:]))

            # ---------------- Phase D1: LSTM layer 1, gates i & f ----------------
            with (
                tc.tile_pool(name="wD1", bufs=1) as wp,
                tc.tile_pool(name="aD1", bufs=1) as apool,
                tc.tile_pool(name="cD1", bufs=2) as cpool,
                tc.tile_pool(name="sD1", bufs=2) as spool,
                tc.tile_pool(name="pD1", bufs=2, space="PSUM") as pp,
            ):
                wis = wp.tile([P, KH * 2 * H], F32, tag="wi1a")
                whs = wp.tile([P, KH * 2 * H], F32, tag="wh1a")
                for k in range(KH):
                    nc.gpsimd.dma_start(
                        out=_r(wis[:, k * 2 * H : (k + 1) * 2 * H]),
                        in_=wi1_d[k, :, 0 : 2 * H],
                    )
                    nc.gpsimd.dma_start(
                        out=_r(whs[:, k * 2 * H : (k + 1) * 2 * H]),
                        in_=wh1_d[k, :, 0 : 2 * H],
                    )
                for i in range(NBLK):
                    x1t = apool.tile([P, KH * N], F32, tag="x1t")
                    hp = apool.tile([P, KH * N], F32, tag="hp1")
                    for k in range(KH):
                        nc.sync.dma_start(
                            out=_r(x1t[:, k * N : (k + 1) * N]), in_=x1_d[i, k]
                        )
                        nc.sync.dma_start(
                            out=_r(hp[:, k * N : (k + 1) * N]), in_=h0l1_d[i, k]
                        )
                    for j in range(KH):
                        outs = []
                        for gi in range(2):  # 0 -> gate i, 1 -> gate f
                            m = gi * KH + j
                            ps = pp.tile([P, N], F32, tag=f"d1g{gi}")
                            for k in range(KH):
                                nc.tensor.matmul(
                                    ps[:],
                                    _r(
                                        wis[
                                            :,
                                            k * 2 * H + m * P : k * 2 * H + (m + 1) * P,
                                        ]
                                    ),
                                    _r(x1t[:, k * N : (k + 1) * N]),
                                    start=(k == 0),
                                    stop=False,
                                )
                            for k in range(KH):
                                nc.tensor.matmul(
                                    ps[:],
                                    _r(
                                        whs[
                                            :,
                                            k * 2 * H + m * P : k * 2 * H + (m + 1) * P,
                                        ]
                                    ),
                                    _r(hp[:, k * N : (k + 1) * N]),
                                    start=False,
                                    stop=(k == KH - 1),
                                )
                            g_sb = spool.tile([P, N], F32, tag=f"d1s{gi}")
                            nc.scalar.activation(
                                g_sb[:], ps[:], AF.Sigmoid, bias=bcol(_C_BG1 + m)
                            )
                            outs.append(g_sb)
                        nc.sync.dma_start(out=si_d[i, j], in_=outs[0][:])
                        ct = cpool.tile([P, N], F32, tag="c1")
                        nc.sync.dma_start(out=_r(ct[:]), in_=c0l1_d[i, j])
                        pf = spool.tile([P, N], F32, tag="pf")
                        nc.vector.tensor_tensor(pf[:], outs[1][:], ct[:], OP.mult)
                        nc.sync.dma_start(out=pfc_d[i, j], in_=pf[:])

            # ---------------- Phase D2: LSTM layer 1 gates g & o, decode ----------------
            with (
                tc.tile_pool(name="wD2", bufs=1) as wp,
                tc.tile_pool(name="aD2", bufs=1) as apool,
                tc.tile_pool(name="ioD2", bufs=2) as iop,
                tc.tile_pool(name="sD2", bufs=2) as spool,
                tc.tile_pool(name="xdD2", bufs=2) as xdp,
                tc.tile_pool(name="pD2", bufs=2, space="PSUM") as pp,
                tc.tile_pool(name="pdec", bufs=2, space="PSUM") as pdec,
            ):
                wgs = wp.tile([P, KH * 2 * H], F32, tag="wi1b")
                whs = wp.tile([P, KH * 2 * H], F32, tag="wh1b")
                for k in range(KH):
                    nc.gpsimd.dma_start(
                        out=_r(wgs[:, k * 2 * H : (k + 1) * 2 * H]),
                        in_=wi1_d[k, :, 2 * H : 4 * H],
                    )
                    nc.gpsimd.dma_start(
                        out=_r(whs[:, k * 2 * H : (k + 1) * 2 * H]),
                        in_=wh1_d[k, :, 2 * H : 4 * H],
                    )
                dws = wp.tile([P, KH * 2], F32, tag="decw")
                for k in range(KH):
                    nc.gpsimd.dma_start(out=_r(dws[:, 2 * k : 2 * k + 2]), in_=decw_d[k])

                for i in range(NBLK):
                    x1t = apool.tile([P, KH * N], F32, tag="x1u")
                    hp = apool.tile([P, KH * N], F32, tag="hp2")
                    for k in range(KH):
                        nc.sync.dma_start(
                            out=_r(x1t[:, k * N : (k + 1) * N]), in_=x1_d[i, k]
                        )
                        nc.sync.dma_start(
                            out=_r(hp[:, k * N : (k + 1) * N]), in_=h0l1_d[i, k]
                        )
                    pd = pdec.tile([2, N], F32, tag="pd")
                    for j in range(KH):
                        go = []
                        for gi, func in [(2, AF.Tanh), (3, AF.Sigmoid)]:
                            m = gi * KH + j
                            mm = m - 2 * KH  # index into wgs/whs
                            ps = pp.tile([P, N], F32, tag=f"d2g{gi}")
                            for k in range(KH):
                                nc.tensor.matmul(
                                    ps[:],
                                    _r(
                                        wgs[
                                            :,
                                            k * 2 * H
                                            + mm * P : k * 2 * H
                                            + (mm + 1) * P,
                                        ]
                                    ),
                                    _r(x1t[:, k * N : (k + 1) * N]),
                                    start=(k == 0),
                                    stop=False,
                                )
                            for k in range(KH):
                                nc.tensor.matmul(
                                    ps[:],
                                    _r(
                                        whs[
                                            :,
                                            k * 2 * H
                                            + mm * P : k * 2 * H
                                            + (mm + 1) * P,
                                        ]
                                    ),
                                    _r(hp[:, k * N : (k + 1) * N]),
                                    start=False,
                                    stop=(k == KH - 1),
                                )
                            g_sb = spool.tile([P, N], F32, tag=f"d2s{gi}")
                            nc.scalar.activation(
                                g_sb[:], ps[:], func, bias=bcol(_C_BG1 + m)
                            )
                            go.append(g_sb)
                        sit = iop.tile([P, N], F32, tag="sit")
                        nc.sync.dma_start(out=sit[:], in_=si_d[i, j])
                        pft = iop.tile([P, N], F32, tag="pft")
                        nc.sync.dma_start(out=pft[:], in_=pfc_d[i, j])
                        # c_new = pfc + si*tg ; h2 = so*tanh(c_new); xdec = tanh(h2)
                        cn = spool.tile([P, N], F32, tag="cn2")
                        nc.vector.tensor_tensor(cn[:], sit[:], go[0][:], OP.mult)
                        nc.vector.tensor_tensor(cn[:], cn[:], pft[:], OP.add)
                        tcn = spool.tile([P, N], F32, tag="tcn2")
                        nc.scalar.activation(tcn[:], cn[:], AF.Tanh)
                        h2 = spool.tile([P, N], F32, tag="h2")
                        nc.vector.tensor_tensor(h2[:], go[1][:], tcn[:], OP.mult)
                        xd = xdp.tile([P, N], F32, tag="xd")
                        nc.scalar.activation(_r(xd[:]), h2[:], AF.Tanh)
                        nc.tensor.matmul(
                            pd[:],
                            _r(dws[:, 2 * j : 2 * j + 2]),
                            _r(xd[:]),
                            start=(j == 0),
                            stop=(j == KH - 1),
                            skip_group_check=True,
                        )
                    x0_t = iop.tile([2, N], F32, tag="x0f")
                    nc.sync.dma_start(out=_r(x0_t[:]), in_=x0_d[i])
                    ob = iop.tile([2, N], F32, tag="ob")
                    # out = x0*k2 + pd   (pd already includes the -k1*k2 factor)
                    nc.vector.scalar_tensor_tensor(
                        ob[:], x0_t[:], bias_sb[0:2, _C_K2 : _C_K2 + 1], pd[:],
                        OP.mult, OP.add,
                    )
                    nc.sync.dma_start(out=out_d[i], in_=ob[:])

    nc.compile()
    return nc


_NC_CACHE = None


def _get_nc():
    global _NC_CACHE
    if _NC_CACHE is None:
        _NC_CACHE = _build_nc()
    return _NC_CACHE


def _prep_maps(noise_start_seq_z, noise_init, alphas, baralphas, params):
    f32 = np.float32
    z = np.asarray(noise_start_seq_z, f32)[:, :, 0]      # (B, 2)
    nin = 0.3 * np.asarray(noise_init, f32)              # (B, 2)

    a = float(np.asarray(alphas)[1])
    ba = float(np.asarray(baralphas)[1])
    k1 = (1.0 - a) / np.sqrt(1.0 - ba)
    k2 = 1.0 / np.sqrt(a)

    pr = params
    g = lambda t: np.ascontiguousarray(np.asarray(t, f32))

    def wt(w, kshape):  # torch (out,in) -> (in,out) tiled (KH,P,out)
        return np.ascontiguousarray(np.asarray(w, f32).T).reshape(kshape)

    w0h = g(pr["init_h"]["w0"]).T.copy()                # (2, 1024)
    w0c = g(pr["init_c"]["w0"]).T.copy()
    w1h = wt(pr["init_h"]["w1"], (KH, P, H))
    w1c = wt(pr["init_c"]["w1"], (KH, P, H))
    w2h = wt(pr["init_h"]["w2"], (KH, P, 2 * H))
    w2c = wt(pr["init_c"]["w2"], (KH, P, 2 * H))
    wi0 = g(pr["lstm"][0]["wi"]).T.copy()               # (2, 4096)
    wh0 = wt(pr["lstm"][0]["wh"], (KH, P, G))
    wi1 = wt(pr["lstm"][1]["wi"], (KH, P, G))
    wh1 = wt(pr["lstm"][1]["wh"], (KH, P, G))
    decw = np.asarray(pr["dec_w"], f32).reshape(H)      # (1024,)
    decw_scaled = (-k1 * k2) * decw
    decw2 = np.repeat(decw_scaled.reshape(KH, P, 1), 2, axis=2).astype(f32)
    decw2 = np.ascontiguousarray(decw2)

    def bias_chunks(b, nch):
        return np.asarray(b, f32).reshape(nch, P).T     # (P, nch)

    biases = np.zeros((P, BIAS_COLS), f32)
    biases[:, _C_B0H:_C_B0H + 8] = bias_chunks(pr["init_h"]["b0"], 8)
    biases[:, _C_B1H:_C_B1H + 8] = bias_chunks(pr["init_h"]["b1"], 8)
    biases[:, _C_B2H:_C_B2H + 16] = bias_chunks(pr["init_h"]["b2"], 16)
    biases[:, _C_B0C:_C_B0C + 8] = bias_chunks(pr["init_c"]["b0"], 8)
    biases[:, _C_B1C:_C_B1C + 8] = bias_chunks(pr["init_c"]["b1"], 8)
    biases[:, _C_B2C:_C_B2C + 16] = bias_chunks(pr["init_c"]["b2"], 16)
    bg0 = np.asarray(pr["lstm"][0]["bi"], f32) + np.asarray(pr["lstm"][0]["bh"], f32)
    bg1 = np.asarray(pr["lstm"][1]["bi"], f32) + np.asarray(pr["lstm"][1]["bh"], f32)
    biases[:, _C_BG0:_C_BG0 + 32] = bias_chunks(bg0, 32)
    biases[:, _C_BG1:_C_BG1 + 32] = bias_chunks(bg1, 32)
    biases[:, _C_ALPHA] = f32(pr["init_h"]["a0"])
    biases[:, _C_ALPHA + 1] = f32(pr["init_h"]["a1"])
    biases[:, _C_ALPHA + 2] = f32(pr["init_c"]["a0"])
    biases[:, _C_ALPHA + 3] = f32(pr["init_c"]["a1"])
    biases[:, _C_K2] = f32(k2)

    shared = {
        "w0h": w0h, "w0c": w0c, "w1h": w1h, "w1c": w1c, "w2h": w2h, "w2c": w2c,
        "wi0": wi0, "wh0": wh0, "wi1": wi1, "wh1": wh1, "decw": decw2,
        "biases": biases,
    }
    in_maps = []
    for c in range(NCORES):
        sl = slice(c * BC, (c + 1) * BC)
        zc = z[sl]                                       # (BC, 2)
        ninc = nin[sl]
        # (NBLK, 2, N): [i, l, n] = zc[i*N+n, l]
        x0 = np.ascontiguousarray(zc.reshape(NBLK, N, 2).transpose(0, 2, 1))
        nn = np.ascontiguousarray(ninc.reshape(NBLK, N, 2).transpose(0, 2, 1))
        in_maps.append({**shared, "x0": x0, "nin": nn})
    return in_maps, z


def kernel(noise_start_seq_z, noise_init, alphas, betas, baralphas, params,
           batch_size, seq_len, dim_seq, num_diffusion_steps, **_unused):
    nc = _get_nc()
    in_maps, z = _prep_maps(noise_start_seq_z, noise_init, alphas, baralphas, params)
    res = run_bass_kernel_spmd(nc, in_maps, list(range(NCORES)))

    out = np.zeros((T_STEPS, B, 2, 1), np.float32)
    out[T_STEPS - 1, :, :, 0] = z
    dec = np.concatenate(
        [res.results[c]["out"].transpose(0, 2, 1).reshape(BC, 2) for c in range(NCORES)],
        axis=0,
    )
    out[T_STEPS - 2, :, :, 0] = dec
    return out
